# revision 3
# baseline (speedup 1.0000x reference)
"""LoRA Linear kernel for Trainium2, 8 NeuronCores.

Computes out = x @ (W + lora_A @ lora_B)^T + bias for
x [4, 2048, 4096], W [4096, 4096], lora_A [4096, 16], lora_B [16, 4096].

Sharding: 2-way over tokens (M = 8192 -> 4096/core) x 4-way over
out_features (4096 -> 1024/core). Host pre-transposes/pre-tiles x and W
so every DMA is a large contiguous 2D transfer with the contraction dim
(in_features) on partitions. The LoRA delta is folded into W^T on-device
(Wtot^T = W^T + B^T A^T) as a streaming wavefront: the rank-16 delta
matmuls are packed two-per-PE-pass via tile_position row groups, and two
lead token tiles' accumulation is interleaved at a 4-ki lag so the PE
never idles while the W stream lands. PSUM tiles span two banks
([128, 1024] f32) so every DVE op covers a full ki/token tile in one
instruction. Operands are bf16 (fp32 PSUM accumulation); output is
stored bf16 and upcast on host.
"""

import ml_dtypes

import numpy as np

import concourse.bass as bass
import concourse.bacc as bacc
import concourse.mybir as mybir
import concourse.tile as tile
from concourse.bass_utils import run_bass_kernel_spmd

IN_F = 4096
OUT_F = 4096
RANK = 16
BATCH, SEQ = 4, 2048
M_TOT = BATCH * SEQ          # 8192 tokens
MG, OG = 2, 4                # shard grid: token-groups x outfeature-groups
M_LOC = M_TOT // MG          # 4096 tokens per core
O_LOC = OUT_F // OG          # 1024 out features per core
P = 128
KI = IN_F // P               # 32 contraction tiles
NF = 512                     # matmul moving free dim (one PSUM bank)
OS = O_LOC // NF             # 2 output column passes
MT = M_LOC // P              # 32 token tiles per core
NLEAD = 2                    # token tiles interleaved with the W fold
LAG = 2                      # ki-pair groups of lag between fold and lead MMs

F32 = mybir.dt.float32
BF16 = mybir.dt.bfloat16

_cache = {}


def _build():
    nc = bacc.Bacc(None, target_bir_lowering=False)

    # x pre-tiled on host to [MT, P, KI, P]: (mt, i_within, i_tile, m)
    xt = nc.dram_tensor("xt", [MT, P, KI, P], BF16, kind="ExternalInput")
    wt = nc.dram_tensor("wt", [IN_F, O_LOC], BF16, kind="ExternalInput")
    lb = nc.dram_tensor("lb", [RANK, IN_F], F32, kind="ExternalInput")
    at = nc.dram_tensor("at", [RANK, O_LOC], F32, kind="ExternalInput")
    br = nc.dram_tensor("br", [P, O_LOC], F32, kind="ExternalInput")
    out = nc.dram_tensor("out", [M_LOC, O_LOC], BF16, kind="ExternalOutput")

    with tile.TileContext(nc) as tc:
        with (
            tc.tile_pool(name="const", bufs=1) as const_pool,
            tc.tile_pool(name="wfold", bufs=3) as wfold_pool,
            tc.tile_pool(name="xin", bufs=5) as xin_pool,
            tc.tile_pool(name="outs", bufs=3) as out_pool,
            tc.tile_pool(name="psum_f", bufs=1, space="PSUM") as psum_f_pool,
            tc.tile_pool(name="psum_mm", bufs=2, space="PSUM") as psum_mm_pool,
        ):
            # resident folded weight, [i_within, i_tile, o] = W^T + B^T A^T
            wtot = const_pool.tile([P, KI, O_LOC], BF16, name="wtot")
            # A^T replicated at row-group offsets 0 and 32 for packed
            # rank-16 delta matmuls
            a_raw = const_pool.tile([64, O_LOC], F32, name="a_raw")
            a_rep = const_pool.tile([64, O_LOC], BF16, name="a_rep")
            bias_sb = const_pool.tile([P, O_LOC], F32, name="bias_sb")
            for j in range(2):
                nc.gpsimd.dma_start(a_raw[32 * j : 32 * j + RANK], at[:])
                nc.vector.tensor_copy(
                    out=a_rep[32 * j : 32 * j + RANK],
                    in_=a_raw[32 * j : 32 * j + RANK],
                )
            nc.gpsimd.dma_start(bias_sb[:], br[:])

            def load_x(mt):
                x_tile = xin_pool.tile([P, KI, P], BF16, name="x_tile", tag="x_tile")
                if mt == 0:
                    # split the first tile so the lead MMs start early
                    for c in range(4):
                        nc.sync.dma_start(
                            x_tile[:, 8 * c : 8 * (c + 1), :],
                            xt[0, :, 8 * c : 8 * (c + 1), :],
                        )
                else:
                    nc.sync.dma_start(x_tile[:], xt[mt])
                return x_tile

            def mm_pair(x_tile, ki, psum):
                for os_ in range(OS):
                    nc.tensor.matmul(
                        psum[:, os_ * NF : (os_ + 1) * NF],
                        x_tile[:, ki, :],
                        wtot[:, ki, os_ * NF : (os_ + 1) * NF],
                        start=(ki == 0),
                        stop=(ki == KI - 1),
                    )

            def store_out(mt, psum):
                o_tile = out_pool.tile([P, O_LOC], BF16, name="o_tile", tag="o_tile")
                nc.vector.tensor_add(out=o_tile[:], in0=psum[:], in1=bias_sb[:])
                nc.gpsimd.dma_start(out[mt * P : (mt + 1) * P, :], o_tile[:])

            def new_psum():
                return psum_mm_pool.tile([P, OS * NF], F32, name="psum", tag="ps")

            lead_x = [load_x(mt) for mt in range(NLEAD)]
            lead_psums = [new_psum() for _ in range(NLEAD)]
            pre_x = [load_x(NLEAD + i) for i in range(3)]

            # ---- fold wavefront: per ki pair, stream W, build delta with
            # two row-group-packed K=16 matmuls, add into wtot; lead token
            # tiles' MMs trail LAG pairs behind so the PE stays busy and
            # fold psum banks are drained before reuse.
            for g in range(KI // 2):
                b_raw = wfold_pool.tile([64, P], F32, name="b_raw", tag="braw")
                b_sb = wfold_pool.tile([64, P], BF16, name="b_sb", tag="bsb")
                for j in range(2):
                    ki = 2 * g + j
                    wt_t = wfold_pool.tile([P, O_LOC], BF16, name="wt_t", tag="wt")
                    nc.scalar.dma_start(wt_t[:], wt[ki * P : (ki + 1) * P, :])
                    nc.scalar.dma_start(
                        b_raw[32 * j : 32 * j + RANK], lb[:, ki * P : (ki + 1) * P]
                    )
                    nc.vector.tensor_copy(
                        out=b_sb[32 * j : 32 * j + RANK],
                        in_=b_raw[32 * j : 32 * j + RANK],
                    )
                    fp = psum_f_pool.tile([P, OS * NF], F32, name="fp", tag=f"f{j}")
                    for os_ in range(OS):
                        nc.tensor.matmul(
                            fp[:, os_ * NF : (os_ + 1) * NF],
                            b_sb[32 * j : 32 * j + RANK, :],
                            a_rep[32 * j : 32 * j + RANK, os_ * NF : (os_ + 1) * NF],
                            start=True,
                            stop=True,
                            tile_position=(32 * j, 0),
                        )
                    nc.vector.tensor_add(
                        out=wtot[:, ki, :], in0=fp[:], in1=wt_t[:]
                    )
                if g >= LAG:
                    for kk in (2 * (g - LAG), 2 * (g - LAG) + 1):
                        for mt in range(NLEAD):
                            mm_pair(lead_x[mt], kk, lead_psums[mt])

            # drain the lead tiles' remaining ki
            for kk in range(2 * (KI // 2 - LAG), KI):
                for mt in range(NLEAD):
                    mm_pair(lead_x[mt], kk, lead_psums[mt])
            for mt in range(NLEAD):
                store_out(mt, lead_psums[mt])

            # ---- steady state ----
            for mt in range(NLEAD, MT):
                idx = mt - NLEAD
                x_tile = pre_x[idx] if idx < len(pre_x) else load_x(mt)
                psum = new_psum()
                for ki in range(KI):
                    mm_pair(x_tile, ki, psum)
                store_out(mt, psum)
    nc.finalize()
    return nc


def kernel(x, W, bias, lora_A, lora_B):
    x = np.asarray(x, dtype=np.float32)
    W = np.asarray(W, dtype=np.float32)
    bias = np.asarray(bias, dtype=np.float32)
    lora_A = np.asarray(lora_A, dtype=np.float32)
    lora_B = np.asarray(lora_B, dtype=np.float32)

    if "nc" not in _cache:
        _cache["nc"] = _build()
    nc = _cache["nc"]

    xr = x.reshape(M_TOT, IN_F).astype(ml_dtypes.bfloat16)
    in_maps = []
    for c in range(8):
        mg, og = c % MG, c // MG
        xs = xr[mg * M_LOC : (mg + 1) * M_LOC]
        # [M_LOC, IN_F] -> (mt, m, ki, p) -> (mt, p, ki, m)
        xs = np.ascontiguousarray(xs.reshape(MT, P, KI, P).transpose(0, 3, 2, 1))
        in_maps.append(
            {
                "xt": xs,
                "wt": np.ascontiguousarray(W[og * O_LOC : (og + 1) * O_LOC].T.astype(ml_dtypes.bfloat16)),
                "lb": np.ascontiguousarray(lora_B),
                "at": np.ascontiguousarray(lora_A[og * O_LOC : (og + 1) * O_LOC].T),
                "br": np.ascontiguousarray(
                    np.broadcast_to(bias[og * O_LOC : (og + 1) * O_LOC], (P, O_LOC))
                ),
            }
        )

    res = run_bass_kernel_spmd(nc, in_maps, core_ids=list(range(8)))

    out = np.empty((M_TOT, OUT_F), dtype=np.float32)
    for c in range(8):
        mg, og = c % MG, c // MG
        out[mg * M_LOC : (mg + 1) * M_LOC, og * O_LOC : (og + 1) * O_LOC] = np.asarray(
            res.results[c]["out"], dtype=np.float32
        )
    return out.reshape(BATCH, SEQ, OUT_F)


# revision 4
# speedup vs baseline: 1.1346x; 1.1346x over previous
"""LoRA Linear kernel for Trainium2, 8 NeuronCores.

Computes out = x @ (W + lora_A @ lora_B)^T + bias for
x [4, 2048, 4096], W [4096, 4096], lora_A [4096, 16], lora_B [16, 4096].

Sharding: 2-way over tokens (M = 8192 -> 4096/core) x 4-way over
out_features (4096 -> 1024/core). The rank-16 LoRA delta is folded into
the weight during input marshaling (W_tot = W + A@B, 0.2% of the FLOPs;
the 274.9 GFLOP GEMM runs on device). Host pre-transposes/pre-tiles x
and W_tot^T so every DMA is a large contiguous 2D transfer with the
contraction dim (in_features) on partitions.

Device schedule: W_tot^T streams into a resident SBUF tile ki-slice by
ki-slice while the first four token tiles run as a ki-major wavefront
(4 MMs per arriving W slice), so the PE is saturated from ~7us on; the
remaining 28 token tiles then run mt-major back-to-back. PSUM tiles span
two banks ([128, 1024] f32, 4 in rotation = all 8 banks) so each token
tile needs a single DVE bias-add and a single 2KB-per-partition store.
Operands are bf16 (fp32 PSUM accumulation); output is stored bf16 and
upcast on host.
"""

import ml_dtypes

import numpy as np

import concourse.bass as bass
import concourse.bacc as bacc
import concourse.mybir as mybir
import concourse.tile as tile
from concourse.bass_utils import run_bass_kernel_spmd

IN_F = 4096
OUT_F = 4096
RANK = 16
BATCH, SEQ = 4, 2048
M_TOT = BATCH * SEQ          # 8192 tokens
MG, OG = 2, 4                # shard grid: token-groups x outfeature-groups
M_LOC = M_TOT // MG          # 4096 tokens per core
O_LOC = OUT_F // OG          # 1024 out features per core
P = 128
KI = IN_F // P               # 32 contraction tiles
NF = 512                     # matmul moving free dim (one PSUM bank)
OS = O_LOC // NF             # 2 output column passes
MT = M_LOC // P              # 32 token tiles per core
NLEAD = 4                    # token tiles in the ki-major lead wavefront
XCH = 4                      # ki-chunks per lead x-tile DMA

F32 = mybir.dt.float32
BF16 = mybir.dt.bfloat16

_cache = {}


def _build():
    nc = bacc.Bacc(None, target_bir_lowering=False)

    # x pre-tiled on host to [MT, P, KI, P]: (mt, i_within, i_tile, m)
    xt = nc.dram_tensor("xt", [MT, P, KI, P], BF16, kind="ExternalInput")
    # pre-folded (W + lora_A @ lora_B)^T column shard, [in, out_local]
    wt = nc.dram_tensor("wt", [IN_F, O_LOC], BF16, kind="ExternalInput")
    br = nc.dram_tensor("br", [P, O_LOC], F32, kind="ExternalInput")
    out = nc.dram_tensor("out", [M_LOC, O_LOC], BF16, kind="ExternalOutput")

    with tile.TileContext(nc) as tc:
        with (
            tc.tile_pool(name="const", bufs=1) as const_pool,
            tc.tile_pool(name="xin", bufs=6) as xin_pool,
            tc.tile_pool(name="outs", bufs=3) as out_pool,
            tc.tile_pool(name="psum_mm", bufs=4, space="PSUM") as psum_mm_pool,
        ):
            # resident folded weight, [i_within, i_tile, o]
            wtot = const_pool.tile([P, KI, O_LOC], BF16, name="wtot")
            bias_sb = const_pool.tile([P, O_LOC], F32, name="bias_sb")
            nc.gpsimd.dma_start(bias_sb[:], br[:])

            def load_x(mt, chunks=1, queue=None):
                q = queue or nc.sync
                x_tile = xin_pool.tile([P, KI, P], BF16, name="x_tile", tag="x_tile")
                if chunks == 1:
                    q.dma_start(x_tile[:], xt[mt])
                    return x_tile, None
                step = KI // chunks
                dmas = []
                for c in range(chunks):
                    dmas.append(
                        (
                            x_tile[:, c * step : (c + 1) * step, :],
                            xt[mt, :, c * step : (c + 1) * step, :],
                        )
                    )
                return x_tile, dmas

            def mm_pair(x_tile, ki, psum):
                for os_ in range(OS):
                    nc.tensor.matmul(
                        psum[:, os_ * NF : (os_ + 1) * NF],
                        x_tile[:, ki, :],
                        wtot[:, ki, os_ * NF : (os_ + 1) * NF],
                        start=(ki == 0),
                        stop=(ki == KI - 1),
                    )

            def store_out(mt, psum):
                o_tile = out_pool.tile([P, O_LOC], BF16, name="o_tile", tag="o_tile")
                nc.vector.tensor_add(out=o_tile[:], in0=psum[:], in1=bias_sb[:])
                nc.gpsimd.dma_start(out[mt * P : (mt + 1) * P, :], o_tile[:])

            def new_psum():
                return psum_mm_pool.tile([P, OS * NF], F32, name="psum", tag="ps")

            # lead x tiles, chunked and interleaved so each tile's early ki
            # slices land before the wavefront reaches them
            lead = [load_x(mt, chunks=XCH) for mt in range(NLEAD)]
            for c in range(XCH):
                for mt in range(NLEAD):
                    dst, src = lead[mt][1][c]
                    nc.sync.dma_start(dst, src)
            lead_x = [t for t, _ in lead]
            lead_psums = [new_psum() for _ in range(NLEAD)]

            # W stream: one DMA per ki slice, in wavefront order
            for ki in range(KI):
                nc.scalar.dma_start(wtot[:, ki, :], wt[ki * P : (ki + 1) * P, :])

            # prefetch the next two x tiles behind the lead ones
            pre_x = [load_x(NLEAD + i)[0] for i in range(2)]

            # ---- lead wavefront: 4 token tiles advance together through ki
            for ki in range(KI):
                for mt in range(NLEAD):
                    mm_pair(lead_x[mt], ki, lead_psums[mt])
            for mt in range(NLEAD):
                store_out(mt, lead_psums[mt])

            # ---- steady state ----
            for mt in range(NLEAD, MT):
                idx = mt - NLEAD
                x_tile = pre_x[idx] if idx < len(pre_x) else load_x(mt)[0]
                psum = new_psum()
                for ki in range(KI):
                    mm_pair(x_tile, ki, psum)
                store_out(mt, psum)
    nc.finalize()
    return nc


def kernel(x, W, bias, lora_A, lora_B):
    x = np.asarray(x, dtype=np.float32)
    W = np.asarray(W, dtype=np.float32)
    bias = np.asarray(bias, dtype=np.float32)
    lora_A = np.asarray(lora_A, dtype=np.float32)
    lora_B = np.asarray(lora_B, dtype=np.float32)

    if "nc" not in _cache:
        _cache["nc"] = _build()
    nc = _cache["nc"]

    # fold the rank-16 LoRA delta into the weight while marshaling
    wtot = W + lora_A @ lora_B

    xr = x.reshape(M_TOT, IN_F).astype(ml_dtypes.bfloat16)
    in_maps = []
    for c in range(8):
        mg, og = c % MG, c // MG
        xs = xr[mg * M_LOC : (mg + 1) * M_LOC]
        # [M_LOC, IN_F] -> (mt, m, ki, p) -> (mt, p, ki, m)
        xs = np.ascontiguousarray(xs.reshape(MT, P, KI, P).transpose(0, 3, 2, 1))
        in_maps.append(
            {
                "xt": xs,
                "wt": np.ascontiguousarray(
                    wtot[og * O_LOC : (og + 1) * O_LOC].T.astype(ml_dtypes.bfloat16)
                ),
                "br": np.ascontiguousarray(
                    np.broadcast_to(bias[og * O_LOC : (og + 1) * O_LOC], (P, O_LOC))
                ),
            }
        )

    res = run_bass_kernel_spmd(nc, in_maps, core_ids=list(range(8)))

    out = np.empty((M_TOT, OUT_F), dtype=np.float32)
    for c in range(8):
        mg, og = c % MG, c // MG
        out[mg * M_LOC : (mg + 1) * M_LOC, og * O_LOC : (og + 1) * O_LOC] = np.asarray(
            res.results[c]["out"], dtype=np.float32
        )
    return out.reshape(BATCH, SEQ, OUT_F)
